# revision 69
# baseline (speedup 1.0000x reference)
# MoE kernel for Trainium2 (8 NeuronCores, dff-sharded / intra-expert tensor
# parallel).
#
# Strategy:
#  - Host: gate logits = x @ gate_w, top-2 + softmax, gather tokens per expert.
#  - Each core gets a 512-wide dff slice of EVERY expert (w1[:, c*512:(c+1)*512],
#    w2[c*512:(c+1)*512, :]) and processes ALL routed tokens on that slice.
#    Every core therefore does identical work: sum_e L_e = 8192 token-slots x
#    64 PE-cycles/slot = 218.5 us of bf16 matmul -- zero load-imbalance waste
#    (vs. expert-per-core, which pays max_e L_e x 512 cycles).
#  - Device per expert-slot, per token-tile g (<=512 tokens):
#    GEMM1 h = gelu(x^T-major @ w1-slice + b1-slice); GEMM2 y_partial = h @ w2-
#    slice.  GEMM1(g) and GEMM2(g-1) are software-pipelined so ACT gelu latency
#    never stalls the PE.  PE is prewarmed with dummy matmuls during the input
#    DMA so HAM is at full clock when real work starts.
#  - Host: sum the 8 partial y's (disjoint dff slices of the same tokens),
#    add b2, scale by gate weights, scatter-add into the output.
import math
from contextlib import ExitStack

import ml_dtypes
import numpy as np

import concourse.bass as bass
import concourse.mybir as mybir
import concourse.tile as tile
from concourse.bass_utils import run_bass_kernel_spmd

D = 1024
DFF = 4096
E = 8
TOP_K = 2
P = 128
KD = D // P        # 8 contraction tiles for GEMM1
S = DFF // 8       # 512 dff columns per core
NF_S = S // P      # 4 f-tiles per expert slot
ND = D // P        # 8 GEMM2 out tiles
T_TILE = 512
WARM_MM = 8        # dummy matmuls to warm the PE/HAM during input DMA
TAIL_T = 176       # size of the final token tile (shrinks the kernel tail)
RAMP = [128, 256]  # leading tile sizes: start compute on minimal DMA
PIPE = 1           # GEMM2(j) runs after GEMM1(j+PIPE): covers gelu latency

BF16 = mybir.dt.bfloat16
F32 = mybir.dt.float32
NP_BF16 = np.dtype(ml_dtypes.bfloat16)

_neff_cache = {}


def _t_sizes(L, small_tail=False, ramp=False):
    """Split L tokens into ceil(L/512) near-equal tiles (sizes sum to L).
    With small_tail, carve a small final tile so the kernel tail only has a
    short GEMM2 + store to drain.  With ramp, lead with small tiles so the
    first matmuls gate on a fraction of the startup DMA."""
    if ramp and L > sum(RAMP) + T_TILE:
        return list(RAMP) + _t_sizes(L - sum(RAMP))
    if small_tail and L > T_TILE:
        return _t_sizes(L - TAIL_T) + [TAIL_T]
    n = max(1, math.ceil(L / T_TILE))
    base, rem = divmod(L, n)
    return [base + 1] * rem + [base] * (n - rem)


def _split_multiwait_json(bir_bytes: bytes) -> bytes:
    """The walrus build in this container rejects instructions carrying more
    than one sync wait (or update). Split extras onto adjacent single-wait
    EventSemaphore carriers on the same engine: program order on the engine
    preserves the semantics exactly."""
    import json as _json

    bir = _json.loads(bir_bytes)
    for fn in bir["functions"]:
        for blk in fn["blocks"]:
            insts = blk.get("instructions", [])
            out = []
            for inst in insts:
                si = inst.get("sync_info")
                if si:
                    waits = si.get("on_wait") or []
                    if len(waits) > 1:
                        for i, w in enumerate(waits[:-1]):
                            out.append({
                                "debug": inst.get("debug", 0),
                                "engine": inst["engine"],
                                "ins": [],
                                "name": f"{inst['name']}_w{i}",
                                "opcode": "EventSemaphore",
                                "outs": [],
                                "sync_info": {"on_update": [], "on_wait": [w]},
                            })
                        si["on_wait"] = [waits[-1]]
                out.append(inst)
                if si:
                    ups = si.get("on_update") or []
                    if len(ups) > 1:
                        for i, u in enumerate(ups[1:]):
                            out.append({
                                "debug": inst.get("debug", 0),
                                "engine": inst["engine"],
                                "ins": [],
                                "name": f"{inst['name']}_u{i}",
                                "opcode": "EventSemaphore",
                                "outs": [],
                                "sync_info": {"on_update": [u], "on_wait": []},
                            })
                        si["on_update"] = [ups[0]]
            blk["instructions"] = out
    return _json.dumps(bir).encode()


def _patch_to_json(nc: bass.Bass) -> bass.Bass:
    orig = nc.to_json_bytes
    nc.to_json_bytes = lambda: _split_multiwait_json(orig())
    return nc


def _x_chunks(slot_tiles):
    """x DMA chunking per slot: slot 0 transfers tile-by-tile (fine-grained
    startup deps); later slots get one whole-slot DMA each -- every HWDGE DMA
    pays a ~2.2us serial completion cost on its ring, so bulk transfers must
    be few and large."""
    out = []
    for s, sizes in enumerate(slot_tiles):
        out.append(list(sizes) if s == 0 else [sum(sizes)])
    return out


def _build_bass(slot_tiles):
    """slot_tiles: list (one entry per active expert slot) of tile-size lists.

    DRAM layouts (host pre-blocks so every DMA reads large contiguous runs):
      xs : [EA*P, KD*LMAX] bf16; row s*P+p holds the slot's x chunks
           back-to-back, each chunk packed kd-major: [KD, chunk_len] flattened.
      w1 : [EA*P, NF_S*KD*P] bf16; row s*P+p, col (f,k,m) = w1slice[k*P+p, f*P+m]
      w2 : [EA*P, ND*NF_S*P] bf16; row s*P+p, col (dd,f,m) = w2slice[f*P+p, dd*P+m]
      b1 : [P, EA*NF_S] f32, pre-transposed on the host (a row-contiguous
           DMA; a device-side "(g p) -> p g" rearrange would cost 4096
           four-byte descriptors and jam the SDMA engines at startup)
    Output:
      y  : [G*P, ND*512] bf16; row g*P+p, col dd*tsz+c = y_partial[tok c, dd*P+p]
    """
    nc = bass.Bass()
    EA = len(slot_tiles)
    gl = []  # (slot, first_of_slot, tsz)
    for s, sizes in enumerate(slot_tiles):
        for i, tsz in enumerate(sizes):
            gl.append((s, i == 0, tsz))
    G = len(gl)
    chunks = _x_chunks(slot_tiles)
    LMAX = max(sum(c) for c in chunks)

    xs_h = nc.dram_tensor("xs", [EA * P, KD * LMAX], BF16, kind="ExternalInput")
    w1_h = nc.dram_tensor("w1", [EA * P, NF_S * KD * P], BF16, kind="ExternalInput")
    w2_h = nc.dram_tensor("w2", [EA * P, ND * NF_S * P], BF16, kind="ExternalInput")
    b1_h = nc.dram_tensor("b1", [P, EA * NF_S], F32, kind="ExternalInput")
    y_h = nc.dram_tensor("y", [G * P, ND * T_TILE], BF16, kind="ExternalOutput")
    warm_h = nc.dram_tensor("warm", [P, 4], F32, kind="ExternalOutput")

    gelu = mybir.ActivationFunctionType.Gelu

    with ExitStack() as ctx:
        tc = ctx.enter_context(tile.TileContext(nc))
        zpool = ctx.enter_context(tc.tile_pool(name="z", bufs=1))
        wpool = ctx.enter_context(tc.tile_pool(name="w", bufs=2))
        xpool = ctx.enter_context(tc.tile_pool(name="x", bufs=2))
        hpool = ctx.enter_context(tc.tile_pool(name="h", bufs=1 + PIPE))
        bpool = ctx.enter_context(tc.tile_pool(name="b", bufs=1))
        ypool = ctx.enter_context(
            tc.tile_pool(name="y", bufs=2 + len(slot_tiles[0])))
        pwarm = ctx.enter_context(tc.tile_pool(name="pw", bufs=1, space="PSUM"))
        ps1 = ctx.enter_context(tc.tile_pool(name="ps1", bufs=2, space="PSUM"))
        ps2 = ctx.enter_context(tc.tile_pool(name="ps2", bufs=2, space="PSUM"))

        # --- PE prewarm: dummy matmuls on zeroed tiles keep the PE busy from
        # ~t=6.5us so the HAM clock gate is at 8/8 when real matmuls start.
        zw = zpool.tile([P, P], BF16, name="zw")
        zx = zpool.tile([P, T_TILE], BF16, name="zx")
        zpool_ws = zpool.tile([P, 4], F32, name="ws")
        nc.vector.memset(zw[:], 0)
        nc.vector.memset(zx[:], 0)
        nc.vector.memset(zpool_ws[:], 0)
        pw = pwarm.tile([P, T_TILE], F32, name="pw")
        for i in range(WARM_MM):
            nc.tensor.matmul(pw[:], zw[:], zx[:],
                             start=(i == 0), stop=(i == WARM_MM - 1))

        def warm_fill(n):
            # Filler matmuls with no data dependencies, dropped between early
            # real phases: where the startup DMA can't yet feed the PE, they
            # turn would-be idle gaps into PE-busy time so the HAM clock gate
            # warms at ~11us and never re-throttles (a >3.4us idle window
            # would halve the PE clock for the next ~7-20us).  128-col bursts
            # keep the overshoot small once data does arrive.
            for i in range(n):
                nc.tensor.matmul(pw[:, :P], zw[:], zx[:, :P],
                                 start=True, stop=True)

        # --- DMA helpers.  Every HWDGE ring (sync=SP, scalar=ACT) executes
        # its DMAs serially with a ~2.2us completion cost each, so:
        # x rides the sync ring (slot-sized bulk DMAs, ramp tiles for slot 0),
        # weights ride the scalar ring, y partial-outputs ride the gpsimd
        # (SWDGE) ring, keeping the three flows from serializing each other.
        def dma_x_chunk(s, coff, clen):
            t = xpool.tile(
                [P, KD * (clen if s == 0 else LMAX)], BF16,
                tag=f"xr{coff}" if s == 0 else "xbig",
                name=f"x{s}_{coff}")
            nc.sync.dma_start(
                t[:, :KD * clen],
                xs_h[s * P:(s + 1) * P, KD * coff:KD * (coff + clen)])
            return t

        def dma_w(s, split):
            # Slot 0 (startup-critical): w1 split in f-halves, w2 in
            # dd-halves so the first GEMM1/GEMM2 gate on 0.5MB each.
            # Later slots: one DMA per tensor to amortize the serial cost.
            W1C = NF_S * KD * P
            W2C = ND * NF_S * P
            n1 = 2 if split else 1
            ws = {"w1": [], "w2": [], "n1": n1}
            for i in range(n1):
                t = wpool.tile([P, W1C // n1], BF16, tag=f"w1_{i}",
                               name=f"w1_{s}_{i}")
                nc.scalar.dma_start(
                    t[:], w1_h[s * P:(s + 1) * P,
                               i * W1C // n1:(i + 1) * W1C // n1])
                ws["w1"].append(t)
            for i in range(n1):
                t = wpool.tile([P, W2C // n1], BF16, tag=f"w2_{i}",
                               name=f"w2_{s}_{i}")
                nc.scalar.dma_start(
                    t[:], w2_h[s * P:(s + 1) * P,
                               i * W2C // n1:(i + 1) * W2C // n1])
                ws["w2"].append(t)
            return ws

        def w1_block(s, f, k):
            ws = w_t[s]
            half = NF_S // ws["n1"]
            t = ws["w1"][f // half]
            i = (f % half) * KD + k
            return t[:, i * P:(i + 1) * P]

        def w2_block(s, dd, f):
            ws = w_t[s]
            half = ND // ws["n1"]
            t = ws["w2"][dd // half]
            i = (dd % half) * NF_S + f
            return t[:, i * P:(i + 1) * P]

        # startup-critical transfers, interleaved across the two HWDGE rings
        x_c = [None] * G   # per-tile: (chunk_tile, chunk_len, offset_in_chunk)
        slot_t0 = []       # token offset of each tile within its slot
        off = 0
        cur = -1
        for s, _, tsz in gl:
            if s != cur:
                cur, off = s, 0
            slot_t0.append(off)
            off += tsz
        w_t = [None] * EA
        co = 0
        for i, clen in enumerate(chunks[0]):
            t = dma_x_chunk(0, co, clen)
            if i == 0:
                w_t[0] = dma_w(0, split=True)
            for g, (s, _, tsz) in enumerate(gl):
                if s == 0 and co <= slot_t0[g] < co + clen:
                    x_c[g] = (t, clen, slot_t0[g] - co)
            co += clen
        # b1 is pre-transposed on the host: a [P, 32] row-contiguous DMA.
        # (A "(g p) -> p g" rearrange here costs 4096 four-byte descriptors,
        # which jams the SDMA engines for ~7us right in the startup window.)
        b1_raw = bpool.tile([P, EA * NF_S], F32, name="b1r")
        nc.gpsimd.dma_start(b1_raw[:], b1_h[:, :])
        # Funnel b1 through an ACT-engine copy: downstream gelus then reach it
        # via same-engine program order instead of an extra semaphore wait.
        b1_t = bpool.tile([P, EA * NF_S], F32, name="b1c")
        nc.scalar.copy(b1_t[:], b1_raw[:])
        # warm output kept tiny and written once from the zero tile so the
        # "warm" DRAM tensor stays a valid output without a data-wait stall
        nc.sync.dma_start(warm_h[:, :], zpool_ws[:])

        # --- main loop: GEMM1(g) then GEMM2(g-1), pipelined so the gelu of
        # tile g's last f-block completes while GEMM2(g-1) occupies the PE.
        h_t = [None] * G
        y_defer = []

        def gemm2(j):
            s, _, tsz = gl[j]
            yst = ypool.tile([P, ND * T_TILE], BF16, tag="yst", name=f"y{j}")
            for dd in range(ND):
                pt2 = ps2.tile([P, T_TILE], F32, tag="ps2", name="pt2")
                for f in range(NF_S):
                    nc.tensor.matmul(
                        pt2[:, :tsz],
                        w2_block(s, dd, f),
                        h_t[j][f][:, :tsz],
                        start=(f == 0),
                        stop=(f == NF_S - 1),
                    )
                nc.vector.tensor_copy(
                    yst[:, dd * tsz:(dd + 1) * tsz], pt2[:, :tsz])
            # y stores ride the gpsimd (SWDGE) ring so they never contend
            # with x loads for a HWDGE ring slot; the final two tiles use the
            # by-then-idle HWDGE rings (shorter completion latency at the
            # tail), the last one split across both rings in parallel.
            if j == G - 1:
                nc.scalar.dma_start(
                    y_h[j * P:(j + 1) * P, :ND * tsz // 2],
                    yst[:, :ND * tsz // 2])
                nc.sync.dma_start(
                    y_h[j * P:(j + 1) * P, ND * tsz // 2:ND * tsz],
                    yst[:, ND * tsz // 2:ND * tsz])
            elif s == 0 and EA > 2:
                # Slot-0 partials are held in SBUF and stored later: their
                # ~1.5MB would otherwise steal SDMA engine time from the
                # startup-critical x/w fill.
                y_defer.append((j, yst, tsz))
            else:
                # Steady-state y stores ride the gpsimd (SWDGE) ring: its
                # sequencer waits never block the x/w prefetch issues on the
                # HWDGE rings (routing them there measured a 50us regression
                # -- each y's data-wait stalls the ring's later loads).  The
                # last few stores move to the scalar ring, whose load queue
                # is empty by then: the SWDGE queue's serial completions
                # otherwise lag past the final matmul and gate the drain.
                if j >= G - 6:
                    eng = nc.scalar if (G - 1 - j) % 2 else nc.sync
                else:
                    eng = nc.gpsimd
                eng.dma_start(
                    y_h[j * P:(j + 1) * P, :ND * tsz], yst[:, :ND * tsz])
            h_t[j] = None

        slot_ti = []
        cur = -1
        for s, _, _ in gl:
            if s != cur:
                cur, ti = s, 0
            slot_ti.append(ti)
            ti += 1
        for g, (s, first, tsz) in enumerate(gl):
            if first and s == 2:
                for (jd, ystd, tszd) in y_defer:
                    nc.gpsimd.dma_start(
                        y_h[jd * P:(jd + 1) * P, :ND * tszd],
                        ystd[:, :ND * tszd])
                y_defer = []
            if first and s + 1 < EA:
                # One whole-slot x DMA for the next slot, issued a full slot
                # (~25us) ahead of first use.
                t = dma_x_chunk(s + 1, 0, sum(slot_tiles[s + 1]))
                for g2, (s2, _, tsz2) in enumerate(gl):
                    if s2 == s + 1:
                        x_c[g2] = (t, sum(slot_tiles[s + 1]), slot_t0[g2])
            hs = [hpool.tile([P, T_TILE], BF16, tag=f"h{f}", name=f"h{g}_{f}")
                  for f in range(NF_S)]
            xt, clen, xo = x_c[g]
            for f in range(NF_S):
                pt = ps1.tile([P, T_TILE], F32, tag="ps1", name="pt1")
                for k in range(KD):
                    nc.tensor.matmul(
                        pt[:, :tsz],
                        w1_block(s, f, k),
                        xt[:, k * clen + xo:k * clen + xo + tsz],
                        start=(k == 0),
                        stop=(k == KD - 1),
                    )
                nc.scalar.activation(
                    hs[f][:, :tsz], pt[:, :tsz], gelu,
                    bias=b1_t[:, s * NF_S + f:s * NF_S + f + 1],
                )
                wlead = min(1, len(slot_tiles[s]) - 1)
                if f == 0 and slot_ti[g] == wlead and s + 1 < EA:
                    # Emit the next expert slot's weight loads behind the
                    # SECOND tile's gelu: the ACT ring is FIFO, so the 2MB
                    # weight transfer only starts once this slot's x tiles
                    # have cleared -- it can't starve the startup-critical
                    # fill, and still lands a full slot (~20us) early.
                    w_t[s + 1] = dma_w(s + 1, split=False)
            h_t[g] = hs
            if g < 4:
                warm_fill((10, 10, 6, 4)[g])
            if g >= PIPE:
                gemm2(g - PIPE)
                if g - PIPE < 3:
                    warm_fill((8, 6, 4)[g - PIPE])
        for j in range(max(0, G - PIPE), G):
            gemm2(j)
    return _patch_to_json(nc)


def _route(xf: np.ndarray, gate_w: np.ndarray):
    """Top-2 gating identical to the reference (argmax ties -> lower index)."""
    N = xf.shape[0]
    logits = xf @ gate_w  # (N, E) f32
    rows = np.arange(N)
    i1 = logits.argmax(1)
    v1 = logits[rows, i1]
    masked = logits.copy()
    masked[rows, i1] = -np.inf
    i2 = masked.argmax(1)
    v2 = masked[rows, i2]
    # softmax over the two selected logits (v1 >= v2)
    e = np.exp((v2 - v1).astype(np.float32))
    wt1 = (1.0 / (1.0 + e)).astype(np.float32)
    wt2 = (e / (1.0 + e)).astype(np.float32)
    idx_e, wts_e = [], []
    for ex in range(E):
        s1 = np.nonzero(i1 == ex)[0]
        s2 = np.nonzero(i2 == ex)[0]
        idx_e.append(np.concatenate([s1, s2]))
        wts_e.append(np.concatenate([wt1[s1], wt2[s2]]).astype(np.float32))
    return idx_e, wts_e


def kernel(x, gate_w, w1, b1, w2, b2, _trace=False):
    B, T, D_ = x.shape
    N = B * T
    xf = np.ascontiguousarray(x.reshape(N, D_).astype(np.float32))
    idx_e, wts_e = _route(xf, gate_w.astype(np.float32))
    cnts = np.array([len(i) for i in idx_e])
    order = np.argsort(-cnts, kind="stable")
    order = [int(e) for e in order if cnts[e] > 0]
    slot_tiles = [_t_sizes(int(cnts[e]), small_tail=(s == len(order) - 1),
                           ramp=(s == 0))
                  for s, e in enumerate(order)]
    EA = len(order)
    gl = []  # (slot, t0_within_expert, tsz)
    for s, sizes in enumerate(slot_tiles):
        t0 = 0
        for tsz in sizes:
            gl.append((s, t0, tsz))
            t0 += tsz
    G = len(gl)

    key = tuple(tuple(st) for st in slot_tiles)
    if key in _neff_cache:
        nc = _neff_cache[key]
    else:
        nc = _build_bass(slot_tiles)
        _neff_cache[key] = nc

    # --- host-side pre-blocking (shared across cores for xs, per-core for w)
    chunks = _x_chunks(slot_tiles)
    LMAX = max(sum(c) for c in chunks)
    xs = np.zeros((EA * P, KD * LMAX), NP_BF16)
    xg_by_slot = [xf[idx_e[e]] for e in order]
    for s, cl in enumerate(chunks):
        co = 0
        for clen in cl:
            blk = xg_by_slot[s][co:co + clen]                  # [clen, D] f32
            b3 = blk.T.reshape(KD, P, clen).transpose(1, 0, 2)  # [P, KD, clen]
            xs[s * P:(s + 1) * P, KD * co:KD * (co + clen)] = (
                b3.reshape(P, KD * clen).astype(NP_BF16))
            co += clen

    in_maps = []
    for c in range(8):
        cS = c * S
        w1s = np.empty((EA * P, NF_S * KD * P), NP_BF16)
        w2s = np.empty((EA * P, NF_S * ND * P), NP_BF16)
        b1s = np.empty((P, EA * NF_S), np.float32)
        for s, e in enumerate(order):
            a = w1[e][:, cS:cS + S]                          # [D, S]
            w1s[s * P:(s + 1) * P] = (
                a.reshape(KD, P, NF_S, P).transpose(1, 2, 0, 3)
                .reshape(P, NF_S * KD * P).astype(NP_BF16))
            bslc = w2[e][cS:cS + S, :]                       # [S, D]
            w2s[s * P:(s + 1) * P] = (
                bslc.reshape(NF_S, P, ND, P).transpose(1, 2, 0, 3)
                .reshape(P, ND * NF_S * P).astype(NP_BF16))
            b1s[:, s * NF_S:(s + 1) * NF_S] = (
                b1[e][cS:cS + S].reshape(NF_S, P).T)
        in_maps.append({
            "xs": xs,
            "w1": w1s,
            "w2": w2s,
            "b1": np.ascontiguousarray(b1s),
        })

    res = run_bass_kernel_spmd(nc, in_maps, core_ids=list(range(8)),
                               trace=_trace)
    if _trace:
        print(f"HW exec time: {res.exec_time_ns} ns")

    # --- unshard: sum the 8 dff-slice partials, then combine + scatter-add
    ysum = np.zeros((G * P, ND * T_TILE), np.float32)
    for c in range(8):
        ysum += res.results[c]["y"].astype(np.float32)

    out = np.zeros((N, D), np.float32)
    for s, e in enumerate(order):
        L = int(cnts[e])
        ye = np.empty((L, D), np.float32)
        for g, (sg, t0, tsz) in enumerate(gl):
            if sg != s:
                continue
            blk = ysum[g * P:(g + 1) * P, :ND * tsz]
            ye[t0:t0 + tsz] = (
                blk.reshape(P, ND, tsz).transpose(2, 1, 0).reshape(tsz, D))
        yv = ye + b2[e][None, :].astype(np.float32)
        out[idx_e[e]] += wts_e[e][:, None] * yv
    return out.reshape(B, T, D_)


# revision 70
# speedup vs baseline: 1.0008x; 1.0008x over previous
# MoE kernel for Trainium2 (8 NeuronCores, dff-sharded / intra-expert tensor
# parallel).
#
# Strategy:
#  - Host: gate logits = x @ gate_w, top-2 + softmax, gather tokens per expert.
#  - Each core gets a 512-wide dff slice of EVERY expert (w1[:, c*512:(c+1)*512],
#    w2[c*512:(c+1)*512, :]) and processes ALL routed tokens on that slice.
#    Every core therefore does identical work: sum_e L_e = 8192 token-slots x
#    64 PE-cycles/slot = 218.5 us of bf16 matmul -- zero load-imbalance waste
#    (vs. expert-per-core, which pays max_e L_e x 512 cycles).
#  - Device per expert-slot, per token-tile g (<=512 tokens):
#    GEMM1 h = gelu(x^T-major @ w1-slice + b1-slice); GEMM2 y_partial = h @ w2-
#    slice.  GEMM1(g) and GEMM2(g-1) are software-pipelined so ACT gelu latency
#    never stalls the PE.  PE is prewarmed with dummy matmuls during the input
#    DMA so HAM is at full clock when real work starts.
#  - Host: sum the 8 partial y's (disjoint dff slices of the same tokens),
#    add b2, scale by gate weights, scatter-add into the output.
import math
from contextlib import ExitStack

import ml_dtypes
import numpy as np

import concourse.bass as bass
import concourse.mybir as mybir
import concourse.tile as tile
from concourse.bass_utils import run_bass_kernel_spmd

D = 1024
DFF = 4096
E = 8
TOP_K = 2
P = 128
KD = D // P        # 8 contraction tiles for GEMM1
S = DFF // 8       # 512 dff columns per core
NF_S = S // P      # 4 f-tiles per expert slot
ND = D // P        # 8 GEMM2 out tiles
T_TILE = 512
WARM_MM = 8        # dummy matmuls to warm the PE/HAM during input DMA
TAIL_T = 176       # size of the final token tile (shrinks the kernel tail)
RAMP = [128, 256]  # leading tile sizes: start compute on minimal DMA
PIPE = 1           # GEMM2(j) runs after GEMM1(j+PIPE): covers gelu latency

BF16 = mybir.dt.bfloat16
F32 = mybir.dt.float32
NP_BF16 = np.dtype(ml_dtypes.bfloat16)

_neff_cache = {}


def _t_sizes(L, small_tail=False, ramp=False):
    """Split L tokens into ceil(L/512) near-equal tiles (sizes sum to L).
    With small_tail, carve a small final tile so the kernel tail only has a
    short GEMM2 + store to drain.  With ramp, lead with small tiles so the
    first matmuls gate on a fraction of the startup DMA."""
    if ramp and L > sum(RAMP) + T_TILE:
        return list(RAMP) + _t_sizes(L - sum(RAMP))
    if small_tail and L > T_TILE:
        return _t_sizes(L - TAIL_T) + [TAIL_T]
    n = max(1, math.ceil(L / T_TILE))
    base, rem = divmod(L, n)
    return [base + 1] * rem + [base] * (n - rem)


def _split_multiwait_json(bir_bytes: bytes) -> bytes:
    """The walrus build in this container rejects instructions carrying more
    than one sync wait (or update). Split extras onto adjacent single-wait
    EventSemaphore carriers on the same engine: program order on the engine
    preserves the semantics exactly."""
    import json as _json

    bir = _json.loads(bir_bytes)
    for fn in bir["functions"]:
        for blk in fn["blocks"]:
            insts = blk.get("instructions", [])
            out = []
            for inst in insts:
                si = inst.get("sync_info")
                if si:
                    waits = si.get("on_wait") or []
                    if len(waits) > 1:
                        for i, w in enumerate(waits[:-1]):
                            out.append({
                                "debug": inst.get("debug", 0),
                                "engine": inst["engine"],
                                "ins": [],
                                "name": f"{inst['name']}_w{i}",
                                "opcode": "EventSemaphore",
                                "outs": [],
                                "sync_info": {"on_update": [], "on_wait": [w]},
                            })
                        si["on_wait"] = [waits[-1]]
                out.append(inst)
                if si:
                    ups = si.get("on_update") or []
                    if len(ups) > 1:
                        for i, u in enumerate(ups[1:]):
                            out.append({
                                "debug": inst.get("debug", 0),
                                "engine": inst["engine"],
                                "ins": [],
                                "name": f"{inst['name']}_u{i}",
                                "opcode": "EventSemaphore",
                                "outs": [],
                                "sync_info": {"on_update": [u], "on_wait": []},
                            })
                        si["on_update"] = [ups[0]]
            blk["instructions"] = out
    return _json.dumps(bir).encode()


def _patch_to_json(nc: bass.Bass) -> bass.Bass:
    orig = nc.to_json_bytes
    nc.to_json_bytes = lambda: _split_multiwait_json(orig())
    return nc


def _x_chunks(slot_tiles):
    """x DMA chunking per slot: slot 0 transfers tile-by-tile (fine-grained
    startup deps); later slots get one whole-slot DMA each -- every HWDGE DMA
    pays a ~2.2us serial completion cost on its ring, so bulk transfers must
    be few and large."""
    out = []
    for s, sizes in enumerate(slot_tiles):
        out.append(list(sizes) if s == 0 else [sum(sizes)])
    return out


def _build_bass(slot_tiles):
    """slot_tiles: list (one entry per active expert slot) of tile-size lists.

    DRAM layouts (host pre-blocks so every DMA reads large contiguous runs):
      xs : [EA*P, KD*LMAX] bf16; row s*P+p holds the slot's x chunks
           back-to-back, each chunk packed kd-major: [KD, chunk_len] flattened.
      w1 : [EA*P, NF_S*KD*P] bf16; row s*P+p, col (f,k,m) = w1slice[k*P+p, f*P+m]
      w2 : [EA*P, ND*NF_S*P] bf16; row s*P+p, col (dd,f,m) = w2slice[f*P+p, dd*P+m]
      b1 : [P, EA*NF_S] f32, pre-transposed on the host (a row-contiguous
           DMA; a device-side "(g p) -> p g" rearrange would cost 4096
           four-byte descriptors and jam the SDMA engines at startup)
    Output:
      y  : [G*P, ND*512] bf16; row g*P+p, col dd*tsz+c = y_partial[tok c, dd*P+p]
    """
    nc = bass.Bass()
    EA = len(slot_tiles)
    gl = []  # (slot, first_of_slot, tsz)
    for s, sizes in enumerate(slot_tiles):
        for i, tsz in enumerate(sizes):
            gl.append((s, i == 0, tsz))
    G = len(gl)
    chunks = _x_chunks(slot_tiles)
    LMAX = max(sum(c) for c in chunks)

    xs_h = nc.dram_tensor("xs", [EA * P, KD * LMAX], BF16, kind="ExternalInput")
    w1_h = nc.dram_tensor("w1", [EA * P, NF_S * KD * P], BF16, kind="ExternalInput")
    w2_h = nc.dram_tensor("w2", [EA * P, ND * NF_S * P], BF16, kind="ExternalInput")
    b1_h = nc.dram_tensor("b1", [P, EA * NF_S], F32, kind="ExternalInput")
    y_h = nc.dram_tensor("y", [G * P, ND * T_TILE], BF16, kind="ExternalOutput")
    warm_h = nc.dram_tensor("warm", [P, 4], F32, kind="ExternalOutput")

    gelu = mybir.ActivationFunctionType.Gelu

    with ExitStack() as ctx:
        tc = ctx.enter_context(tile.TileContext(nc))
        zpool = ctx.enter_context(tc.tile_pool(name="z", bufs=1))
        wpool = ctx.enter_context(tc.tile_pool(name="w", bufs=2))
        xpool = ctx.enter_context(tc.tile_pool(name="x", bufs=2))
        hpool = ctx.enter_context(tc.tile_pool(name="h", bufs=1 + PIPE))
        bpool = ctx.enter_context(tc.tile_pool(name="b", bufs=1))
        ypool = ctx.enter_context(
            tc.tile_pool(name="y", bufs=2 + len(slot_tiles[0])))
        pwarm = ctx.enter_context(tc.tile_pool(name="pw", bufs=1, space="PSUM"))
        ps1 = ctx.enter_context(tc.tile_pool(name="ps1", bufs=2, space="PSUM"))
        ps2 = ctx.enter_context(tc.tile_pool(name="ps2", bufs=2, space="PSUM"))

        # --- PE prewarm: dummy matmuls on zeroed tiles keep the PE busy from
        # ~t=6.5us so the HAM clock gate is at 8/8 when real matmuls start.
        zw = zpool.tile([P, P], BF16, name="zw")
        zx = zpool.tile([P, T_TILE], BF16, name="zx")
        zpool_ws = zpool.tile([P, 4], F32, name="ws")
        nc.vector.memset(zw[:], 0)
        nc.vector.memset(zx[:], 0)
        nc.vector.memset(zpool_ws[:], 0)
        pw = pwarm.tile([P, T_TILE], F32, name="pw")
        for i in range(WARM_MM):
            nc.tensor.matmul(pw[:], zw[:], zx[:],
                             start=(i == 0), stop=(i == WARM_MM - 1))

        def warm_fill(n):
            # Filler matmuls with no data dependencies, dropped between early
            # real phases: where the startup DMA can't yet feed the PE, they
            # turn would-be idle gaps into PE-busy time so the HAM clock gate
            # warms at ~11us and never re-throttles (a >3.4us idle window
            # would halve the PE clock for the next ~7-20us).  128-col bursts
            # keep the overshoot small once data does arrive.
            for i in range(n):
                nc.tensor.matmul(pw[:, :P], zw[:], zx[:, :P],
                                 start=True, stop=True)

        # --- DMA helpers.  Every HWDGE ring (sync=SP, scalar=ACT) executes
        # its DMAs serially with a ~2.2us completion cost each, so:
        # x rides the sync ring (slot-sized bulk DMAs, ramp tiles for slot 0),
        # weights ride the scalar ring, y partial-outputs ride the gpsimd
        # (SWDGE) ring, keeping the three flows from serializing each other.
        def dma_x_chunk(s, coff, clen):
            t = xpool.tile(
                [P, KD * (clen if s == 0 else LMAX)], BF16,
                tag=f"xr{coff}" if s == 0 else "xbig",
                name=f"x{s}_{coff}")
            nc.sync.dma_start(
                t[:, :KD * clen],
                xs_h[s * P:(s + 1) * P, KD * coff:KD * (coff + clen)])
            return t

        def dma_w(s, split):
            # Slot 0 (startup-critical): w1 split in f-halves, w2 in
            # dd-halves so the first GEMM1/GEMM2 gate on 0.5MB each.
            # Later slots: one DMA per tensor to amortize the serial cost.
            W1C = NF_S * KD * P
            W2C = ND * NF_S * P
            n1 = 2 if split else 1
            ws = {"w1": [], "w2": [], "n1": n1}
            for i in range(n1):
                t = wpool.tile([P, W1C // n1], BF16, tag=f"w1_{i}",
                               name=f"w1_{s}_{i}")
                nc.scalar.dma_start(
                    t[:], w1_h[s * P:(s + 1) * P,
                               i * W1C // n1:(i + 1) * W1C // n1])
                ws["w1"].append(t)
            for i in range(n1):
                t = wpool.tile([P, W2C // n1], BF16, tag=f"w2_{i}",
                               name=f"w2_{s}_{i}")
                nc.scalar.dma_start(
                    t[:], w2_h[s * P:(s + 1) * P,
                               i * W2C // n1:(i + 1) * W2C // n1])
                ws["w2"].append(t)
            return ws

        def w1_block(s, f, k):
            ws = w_t[s]
            half = NF_S // ws["n1"]
            t = ws["w1"][f // half]
            i = (f % half) * KD + k
            return t[:, i * P:(i + 1) * P]

        def w2_block(s, dd, f):
            ws = w_t[s]
            half = ND // ws["n1"]
            t = ws["w2"][dd // half]
            i = (dd % half) * NF_S + f
            return t[:, i * P:(i + 1) * P]

        # startup-critical transfers, interleaved across the two HWDGE rings
        x_c = [None] * G   # per-tile: (chunk_tile, chunk_len, offset_in_chunk)
        slot_t0 = []       # token offset of each tile within its slot
        off = 0
        cur = -1
        for s, _, tsz in gl:
            if s != cur:
                cur, off = s, 0
            slot_t0.append(off)
            off += tsz
        w_t = [None] * EA
        co = 0
        for i, clen in enumerate(chunks[0]):
            t = dma_x_chunk(0, co, clen)
            if i == 0:
                w_t[0] = dma_w(0, split=True)
            for g, (s, _, tsz) in enumerate(gl):
                if s == 0 and co <= slot_t0[g] < co + clen:
                    x_c[g] = (t, clen, slot_t0[g] - co)
            co += clen
        # b1 is pre-transposed on the host: a [P, 32] row-contiguous DMA.
        # (A "(g p) -> p g" rearrange here costs 4096 four-byte descriptors,
        # which jams the SDMA engines for ~7us right in the startup window.)
        b1_raw = bpool.tile([P, EA * NF_S], F32, name="b1r")
        nc.gpsimd.dma_start(b1_raw[:], b1_h[:, :])
        # Funnel b1 through an ACT-engine copy: downstream gelus then reach it
        # via same-engine program order instead of an extra semaphore wait.
        b1_t = bpool.tile([P, EA * NF_S], F32, name="b1c")
        nc.scalar.copy(b1_t[:], b1_raw[:])
        # warm output kept tiny and written once from the zero tile so the
        # "warm" DRAM tensor stays a valid output without a data-wait stall
        nc.sync.dma_start(warm_h[:, :], zpool_ws[:])

        # --- main loop: GEMM1(g) then GEMM2(g-1), pipelined so the gelu of
        # tile g's last f-block completes while GEMM2(g-1) occupies the PE.
        h_t = [None] * G
        y_defer = []

        def gemm2(j):
            s, _, tsz = gl[j]
            yst = ypool.tile([P, ND * T_TILE], BF16, tag="yst", name=f"y{j}")
            for dd in range(ND):
                pt2 = ps2.tile([P, T_TILE], F32, tag="ps2", name="pt2")
                for f in range(NF_S):
                    nc.tensor.matmul(
                        pt2[:, :tsz],
                        w2_block(s, dd, f),
                        h_t[j][f][:, :tsz],
                        start=(f == 0),
                        stop=(f == NF_S - 1),
                    )
                nc.vector.tensor_copy(
                    yst[:, dd * tsz:(dd + 1) * tsz], pt2[:, :tsz])
            # y stores ride the gpsimd (SWDGE) ring so they never contend
            # with x loads for a HWDGE ring slot; the final two tiles use the
            # by-then-idle HWDGE rings (shorter completion latency at the
            # tail), the last one split across both rings in parallel.
            if j == G - 1:
                nc.scalar.dma_start(
                    y_h[j * P:(j + 1) * P, :ND * tsz // 2],
                    yst[:, :ND * tsz // 2])
                nc.sync.dma_start(
                    y_h[j * P:(j + 1) * P, ND * tsz // 2:ND * tsz],
                    yst[:, ND * tsz // 2:ND * tsz])
            elif s == 0 and EA > 2:
                # Slot-0 partials are held in SBUF and stored later: their
                # ~1.5MB would otherwise steal SDMA engine time from the
                # startup-critical x/w fill.
                y_defer.append((j, yst, tsz))
            else:
                # Steady-state y stores ride the gpsimd (SWDGE) ring: its
                # sequencer waits never block the x/w prefetch issues on the
                # HWDGE rings (routing them there measured a 50us regression
                # -- each y's data-wait stalls the ring's later loads).  The
                # last few stores move to the scalar ring, whose load queue
                # is empty by then: the SWDGE queue's serial completions
                # otherwise lag past the final matmul and gate the drain.
                if j >= G - 6:
                    eng = nc.scalar if (G - 1 - j) % 2 else nc.sync
                else:
                    eng = nc.gpsimd
                eng.dma_start(
                    y_h[j * P:(j + 1) * P, :ND * tsz], yst[:, :ND * tsz])
            h_t[j] = None

        slot_ti = []
        cur = -1
        for s, _, _ in gl:
            if s != cur:
                cur, ti = s, 0
            slot_ti.append(ti)
            ti += 1
        for g, (s, first, tsz) in enumerate(gl):
            if first and s in (2, 3) and y_defer:
                # Flush in two batches: dumping all deferred stores at once
                # puts ~40us of serial work on the SWDGE queue, which then
                # runs ~35us behind production for the rest of the kernel
                # and its final completion gates the drain barrier.
                nflush = (len(y_defer) + 1) // 2 if s == 2 else len(y_defer)
                for (jd, ystd, tszd) in y_defer[:nflush]:
                    nc.gpsimd.dma_start(
                        y_h[jd * P:(jd + 1) * P, :ND * tszd],
                        ystd[:, :ND * tszd])
                y_defer = y_defer[nflush:]
            if first and s + 1 < EA:
                # One whole-slot x DMA for the next slot, issued a full slot
                # (~25us) ahead of first use.
                t = dma_x_chunk(s + 1, 0, sum(slot_tiles[s + 1]))
                for g2, (s2, _, tsz2) in enumerate(gl):
                    if s2 == s + 1:
                        x_c[g2] = (t, sum(slot_tiles[s + 1]), slot_t0[g2])
            hs = [hpool.tile([P, T_TILE], BF16, tag=f"h{f}", name=f"h{g}_{f}")
                  for f in range(NF_S)]
            xt, clen, xo = x_c[g]
            for f in range(NF_S):
                pt = ps1.tile([P, T_TILE], F32, tag="ps1", name="pt1")
                for k in range(KD):
                    nc.tensor.matmul(
                        pt[:, :tsz],
                        w1_block(s, f, k),
                        xt[:, k * clen + xo:k * clen + xo + tsz],
                        start=(k == 0),
                        stop=(k == KD - 1),
                    )
                nc.scalar.activation(
                    hs[f][:, :tsz], pt[:, :tsz], gelu,
                    bias=b1_t[:, s * NF_S + f:s * NF_S + f + 1],
                )
                wlead = min(1, len(slot_tiles[s]) - 1)
                if f == 0 and slot_ti[g] == wlead and s + 1 < EA:
                    # Emit the next expert slot's weight loads behind the
                    # SECOND tile's gelu: the ACT ring is FIFO, so the 2MB
                    # weight transfer only starts once this slot's x tiles
                    # have cleared -- it can't starve the startup-critical
                    # fill, and still lands a full slot (~20us) early.
                    w_t[s + 1] = dma_w(s + 1, split=False)
            h_t[g] = hs
            if g < 4:
                warm_fill((10, 10, 6, 4)[g])
            if g >= PIPE:
                gemm2(g - PIPE)
                if g - PIPE < 3:
                    warm_fill((8, 6, 4)[g - PIPE])
        for j in range(max(0, G - PIPE), G):
            gemm2(j)
    return _patch_to_json(nc)


def _route(xf: np.ndarray, gate_w: np.ndarray):
    """Top-2 gating identical to the reference (argmax ties -> lower index)."""
    N = xf.shape[0]
    logits = xf @ gate_w  # (N, E) f32
    rows = np.arange(N)
    i1 = logits.argmax(1)
    v1 = logits[rows, i1]
    masked = logits.copy()
    masked[rows, i1] = -np.inf
    i2 = masked.argmax(1)
    v2 = masked[rows, i2]
    # softmax over the two selected logits (v1 >= v2)
    e = np.exp((v2 - v1).astype(np.float32))
    wt1 = (1.0 / (1.0 + e)).astype(np.float32)
    wt2 = (e / (1.0 + e)).astype(np.float32)
    idx_e, wts_e = [], []
    for ex in range(E):
        s1 = np.nonzero(i1 == ex)[0]
        s2 = np.nonzero(i2 == ex)[0]
        idx_e.append(np.concatenate([s1, s2]))
        wts_e.append(np.concatenate([wt1[s1], wt2[s2]]).astype(np.float32))
    return idx_e, wts_e


def kernel(x, gate_w, w1, b1, w2, b2, _trace=False):
    B, T, D_ = x.shape
    N = B * T
    xf = np.ascontiguousarray(x.reshape(N, D_).astype(np.float32))
    idx_e, wts_e = _route(xf, gate_w.astype(np.float32))
    cnts = np.array([len(i) for i in idx_e])
    order = np.argsort(-cnts, kind="stable")
    order = [int(e) for e in order if cnts[e] > 0]
    slot_tiles = [_t_sizes(int(cnts[e]), small_tail=(s == len(order) - 1),
                           ramp=(s == 0))
                  for s, e in enumerate(order)]
    EA = len(order)
    gl = []  # (slot, t0_within_expert, tsz)
    for s, sizes in enumerate(slot_tiles):
        t0 = 0
        for tsz in sizes:
            gl.append((s, t0, tsz))
            t0 += tsz
    G = len(gl)

    key = tuple(tuple(st) for st in slot_tiles)
    if key in _neff_cache:
        nc = _neff_cache[key]
    else:
        nc = _build_bass(slot_tiles)
        _neff_cache[key] = nc

    # --- host-side pre-blocking (shared across cores for xs, per-core for w)
    chunks = _x_chunks(slot_tiles)
    LMAX = max(sum(c) for c in chunks)
    xs = np.zeros((EA * P, KD * LMAX), NP_BF16)
    xg_by_slot = [xf[idx_e[e]] for e in order]
    for s, cl in enumerate(chunks):
        co = 0
        for clen in cl:
            blk = xg_by_slot[s][co:co + clen]                  # [clen, D] f32
            b3 = blk.T.reshape(KD, P, clen).transpose(1, 0, 2)  # [P, KD, clen]
            xs[s * P:(s + 1) * P, KD * co:KD * (co + clen)] = (
                b3.reshape(P, KD * clen).astype(NP_BF16))
            co += clen

    in_maps = []
    for c in range(8):
        cS = c * S
        w1s = np.empty((EA * P, NF_S * KD * P), NP_BF16)
        w2s = np.empty((EA * P, NF_S * ND * P), NP_BF16)
        b1s = np.empty((P, EA * NF_S), np.float32)
        for s, e in enumerate(order):
            a = w1[e][:, cS:cS + S]                          # [D, S]
            w1s[s * P:(s + 1) * P] = (
                a.reshape(KD, P, NF_S, P).transpose(1, 2, 0, 3)
                .reshape(P, NF_S * KD * P).astype(NP_BF16))
            bslc = w2[e][cS:cS + S, :]                       # [S, D]
            w2s[s * P:(s + 1) * P] = (
                bslc.reshape(NF_S, P, ND, P).transpose(1, 2, 0, 3)
                .reshape(P, ND * NF_S * P).astype(NP_BF16))
            b1s[:, s * NF_S:(s + 1) * NF_S] = (
                b1[e][cS:cS + S].reshape(NF_S, P).T)
        in_maps.append({
            "xs": xs,
            "w1": w1s,
            "w2": w2s,
            "b1": np.ascontiguousarray(b1s),
        })

    res = run_bass_kernel_spmd(nc, in_maps, core_ids=list(range(8)),
                               trace=_trace)
    if _trace:
        print(f"HW exec time: {res.exec_time_ns} ns")

    # --- unshard: sum the 8 dff-slice partials, then combine + scatter-add
    ysum = np.zeros((G * P, ND * T_TILE), np.float32)
    for c in range(8):
        ysum += res.results[c]["y"].astype(np.float32)

    out = np.zeros((N, D), np.float32)
    for s, e in enumerate(order):
        L = int(cnts[e])
        ye = np.empty((L, D), np.float32)
        for g, (sg, t0, tsz) in enumerate(gl):
            if sg != s:
                continue
            blk = ysum[g * P:(g + 1) * P, :ND * tsz]
            ye[t0:t0 + tsz] = (
                blk.reshape(P, ND, tsz).transpose(2, 1, 0).reshape(tsz, D))
        yv = ye + b2[e][None, :].astype(np.float32)
        out[idx_e[e]] += wts_e[e][:, None] * yv
    return out.reshape(B, T, D_)


# revision 71
# speedup vs baseline: 1.0127x; 1.0120x over previous
# MoE kernel for Trainium2 (8 NeuronCores, dff-sharded / intra-expert tensor
# parallel).
#
# Strategy:
#  - Host: gate logits = x @ gate_w, top-2 + softmax, gather tokens per expert.
#  - Each core gets a 512-wide dff slice of EVERY expert (w1[:, c*512:(c+1)*512],
#    w2[c*512:(c+1)*512, :]) and processes ALL routed tokens on that slice.
#    Every core therefore does identical work: sum_e L_e = 8192 token-slots x
#    64 PE-cycles/slot = 218.5 us of bf16 matmul -- zero load-imbalance waste
#    (vs. expert-per-core, which pays max_e L_e x 512 cycles).
#  - Device per expert-slot, per token-tile g (<=512 tokens):
#    GEMM1 h = gelu(x^T-major @ w1-slice + b1-slice); GEMM2 y_partial = h @ w2-
#    slice.  GEMM1(g) and GEMM2(g-1) are software-pipelined so ACT gelu latency
#    never stalls the PE.  PE is prewarmed with dummy matmuls during the input
#    DMA so HAM is at full clock when real work starts.
#  - Host: sum the 8 partial y's (disjoint dff slices of the same tokens),
#    add b2, scale by gate weights, scatter-add into the output.
import math
from contextlib import ExitStack

import ml_dtypes
import numpy as np

import concourse.bass as bass
import concourse.mybir as mybir
import concourse.tile as tile
from concourse.bass_utils import run_bass_kernel_spmd

D = 1024
DFF = 4096
E = 8
TOP_K = 2
P = 128
KD = D // P        # 8 contraction tiles for GEMM1
S = DFF // 8       # 512 dff columns per core
NF_S = S // P      # 4 f-tiles per expert slot
ND = D // P        # 8 GEMM2 out tiles
T_TILE = 512
WARM_MM = 8        # dummy matmuls to warm the PE/HAM during input DMA
TAIL_T = 176       # size of the final token tile (shrinks the kernel tail)
RAMP = [128, 256]  # leading tile sizes: start compute on minimal DMA
PIPE = 1           # GEMM2(j) runs after GEMM1(j+PIPE): covers gelu latency

BF16 = mybir.dt.bfloat16
F32 = mybir.dt.float32
NP_BF16 = np.dtype(ml_dtypes.bfloat16)

_neff_cache = {}


def _t_sizes(L, small_tail=False, ramp=False):
    """Split L tokens into ceil(L/512) near-equal tiles (sizes sum to L).
    With small_tail, carve a small final tile so the kernel tail only has a
    short GEMM2 + store to drain.  With ramp, lead with small tiles so the
    first matmuls gate on a fraction of the startup DMA."""
    if ramp and L > sum(RAMP) + T_TILE:
        return list(RAMP) + _t_sizes(L - sum(RAMP))
    if small_tail and L > T_TILE:
        return _t_sizes(L - TAIL_T) + [TAIL_T]
    n = max(1, math.ceil(L / T_TILE))
    base, rem = divmod(L, n)
    return [base + 1] * rem + [base] * (n - rem)


def _split_multiwait_json(bir_bytes: bytes) -> bytes:
    """The walrus build in this container rejects instructions carrying more
    than one sync wait (or update). Split extras onto adjacent single-wait
    EventSemaphore carriers on the same engine: program order on the engine
    preserves the semantics exactly."""
    import json as _json

    bir = _json.loads(bir_bytes)
    for fn in bir["functions"]:
        for blk in fn["blocks"]:
            insts = blk.get("instructions", [])
            out = []
            for inst in insts:
                si = inst.get("sync_info")
                if si:
                    waits = si.get("on_wait") or []
                    if len(waits) > 1:
                        for i, w in enumerate(waits[:-1]):
                            out.append({
                                "debug": inst.get("debug", 0),
                                "engine": inst["engine"],
                                "ins": [],
                                "name": f"{inst['name']}_w{i}",
                                "opcode": "EventSemaphore",
                                "outs": [],
                                "sync_info": {"on_update": [], "on_wait": [w]},
                            })
                        si["on_wait"] = [waits[-1]]
                out.append(inst)
                if si:
                    ups = si.get("on_update") or []
                    if len(ups) > 1:
                        for i, u in enumerate(ups[1:]):
                            out.append({
                                "debug": inst.get("debug", 0),
                                "engine": inst["engine"],
                                "ins": [],
                                "name": f"{inst['name']}_u{i}",
                                "opcode": "EventSemaphore",
                                "outs": [],
                                "sync_info": {"on_update": [u], "on_wait": []},
                            })
                        si["on_update"] = [ups[0]]
            blk["instructions"] = out
    return _json.dumps(bir).encode()


def _patch_to_json(nc: bass.Bass) -> bass.Bass:
    orig = nc.to_json_bytes
    nc.to_json_bytes = lambda: _split_multiwait_json(orig())
    return nc


def _x_chunks(slot_tiles):
    """x DMA chunking per slot: slot 0 transfers tile-by-tile (fine-grained
    startup deps); later slots get one whole-slot DMA each -- every HWDGE DMA
    pays a ~2.2us serial completion cost on its ring, so bulk transfers must
    be few and large."""
    out = []
    for s, sizes in enumerate(slot_tiles):
        out.append(list(sizes) if s == 0 else [sum(sizes)])
    return out


def _build_bass(slot_tiles):
    """slot_tiles: list (one entry per active expert slot) of tile-size lists.

    DRAM layouts (host pre-blocks so every DMA reads large contiguous runs):
      xs : [EA*P, KD*LMAX] bf16; row s*P+p holds the slot's x chunks
           back-to-back, each chunk packed kd-major: [KD, chunk_len] flattened.
      w1 : [EA*P, NF_S*KD*P] bf16; row s*P+p, col (f,k,m) = w1slice[k*P+p, f*P+m]
      w2 : [EA*P, ND*NF_S*P] bf16; row s*P+p, col (dd,f,m) = w2slice[f*P+p, dd*P+m]
      b1 : [P, EA*NF_S] f32, pre-transposed on the host (a row-contiguous
           DMA; a device-side "(g p) -> p g" rearrange would cost 4096
           four-byte descriptors and jam the SDMA engines at startup)
    Output:
      y  : [G*P, ND*512] bf16; row g*P+p, col dd*tsz+c = y_partial[tok c, dd*P+p]
    """
    nc = bass.Bass()
    EA = len(slot_tiles)
    gl = []  # (slot, first_of_slot, tsz)
    for s, sizes in enumerate(slot_tiles):
        for i, tsz in enumerate(sizes):
            gl.append((s, i == 0, tsz))
    G = len(gl)
    chunks = _x_chunks(slot_tiles)
    LMAX = max(sum(c) for c in chunks)

    xs_h = nc.dram_tensor("xs", [EA * P, KD * LMAX], BF16, kind="ExternalInput")
    w1_h = nc.dram_tensor("w1", [EA * P, NF_S * KD * P], BF16, kind="ExternalInput")
    w2_h = nc.dram_tensor("w2", [EA * P, ND * NF_S * P], BF16, kind="ExternalInput")
    b1_h = nc.dram_tensor("b1", [P, EA * NF_S], F32, kind="ExternalInput")
    y_h = nc.dram_tensor("y", [G * P, ND * T_TILE], BF16, kind="ExternalOutput")
    warm_h = nc.dram_tensor("warm", [P, 4], F32, kind="ExternalOutput")

    gelu = mybir.ActivationFunctionType.Gelu

    with ExitStack() as ctx:
        tc = ctx.enter_context(tile.TileContext(nc))
        zpool = ctx.enter_context(tc.tile_pool(name="z", bufs=1))
        wpool = ctx.enter_context(tc.tile_pool(name="w", bufs=2))
        xpool = ctx.enter_context(tc.tile_pool(name="x", bufs=2))
        hpool = ctx.enter_context(tc.tile_pool(name="h", bufs=1 + PIPE))
        bpool = ctx.enter_context(tc.tile_pool(name="b", bufs=1))
        ypool = ctx.enter_context(
            tc.tile_pool(name="y", bufs=2 + len(slot_tiles[0])))
        pwarm = ctx.enter_context(tc.tile_pool(name="pw", bufs=1, space="PSUM"))
        ps1 = ctx.enter_context(tc.tile_pool(name="ps1", bufs=2, space="PSUM"))
        ps2 = ctx.enter_context(tc.tile_pool(name="ps2", bufs=2, space="PSUM"))

        # --- PE prewarm: dummy matmuls on zeroed tiles keep the PE busy from
        # ~t=6.5us so the HAM clock gate is at 8/8 when real matmuls start.
        zw = zpool.tile([P, P], BF16, name="zw")
        zx = zpool.tile([P, T_TILE], BF16, name="zx")
        zpool_ws = zpool.tile([P, 4], F32, name="ws")
        nc.vector.memset(zw[:], 0)
        nc.vector.memset(zx[:], 0)
        nc.vector.memset(zpool_ws[:], 0)
        pw = pwarm.tile([P, T_TILE], F32, name="pw")
        for i in range(WARM_MM):
            nc.tensor.matmul(pw[:], zw[:], zx[:],
                             start=(i == 0), stop=(i == WARM_MM - 1))

        def warm_fill(n):
            # Filler matmuls with no data dependencies, dropped between early
            # real phases: where the startup DMA can't yet feed the PE, they
            # turn would-be idle gaps into PE-busy time so the HAM clock gate
            # warms at ~11us and never re-throttles (a >3.4us idle window
            # would halve the PE clock for the next ~7-20us).  128-col bursts
            # keep the overshoot small once data does arrive.
            for i in range(n):
                nc.tensor.matmul(pw[:, :P], zw[:], zx[:, :P],
                                 start=True, stop=True)

        # --- DMA helpers.  Every HWDGE ring (sync=SP, scalar=ACT) executes
        # its DMAs serially with a ~2.2us completion cost each, so:
        # x rides the sync ring (slot-sized bulk DMAs, ramp tiles for slot 0),
        # weights ride the scalar ring, y partial-outputs ride the gpsimd
        # (SWDGE) ring, keeping the three flows from serializing each other.
        def dma_x_chunk(s, coff, clen):
            t = xpool.tile(
                [P, KD * (clen if s == 0 else LMAX)], BF16,
                tag=f"xr{coff}" if s == 0 else "xbig",
                name=f"x{s}_{coff}")
            nc.sync.dma_start(
                t[:, :KD * clen],
                xs_h[s * P:(s + 1) * P, KD * coff:KD * (coff + clen)])
            return t

        def dma_w(s, split):
            # Slot 0 (startup-critical): w1 split in f-halves, w2 in
            # dd-halves so the first GEMM1/GEMM2 gate on 0.5MB each.
            # Later slots: one DMA per tensor to amortize the serial cost.
            W1C = NF_S * KD * P
            W2C = ND * NF_S * P
            n1 = 2 if split else 1
            ws = {"w1": [], "w2": [], "n1": n1}
            for i in range(n1):
                t = wpool.tile([P, W1C // n1], BF16, tag=f"w1_{i}",
                               name=f"w1_{s}_{i}")
                nc.scalar.dma_start(
                    t[:], w1_h[s * P:(s + 1) * P,
                               i * W1C // n1:(i + 1) * W1C // n1])
                ws["w1"].append(t)
            for i in range(n1):
                t = wpool.tile([P, W2C // n1], BF16, tag=f"w2_{i}",
                               name=f"w2_{s}_{i}")
                nc.scalar.dma_start(
                    t[:], w2_h[s * P:(s + 1) * P,
                               i * W2C // n1:(i + 1) * W2C // n1])
                ws["w2"].append(t)
            return ws

        def w1_block(s, f, k):
            ws = w_t[s]
            half = NF_S // ws["n1"]
            t = ws["w1"][f // half]
            i = (f % half) * KD + k
            return t[:, i * P:(i + 1) * P]

        def w2_block(s, dd, f):
            ws = w_t[s]
            half = ND // ws["n1"]
            t = ws["w2"][dd // half]
            i = (dd % half) * NF_S + f
            return t[:, i * P:(i + 1) * P]

        # startup-critical transfers, interleaved across the two HWDGE rings
        x_c = [None] * G   # per-tile: (chunk_tile, chunk_len, offset_in_chunk)
        slot_t0 = []       # token offset of each tile within its slot
        off = 0
        cur = -1
        for s, _, tsz in gl:
            if s != cur:
                cur, off = s, 0
            slot_t0.append(off)
            off += tsz
        w_t = [None] * EA
        co = 0
        for i, clen in enumerate(chunks[0]):
            t = dma_x_chunk(0, co, clen)
            if i == 0:
                w_t[0] = dma_w(0, split=True)
            for g, (s, _, tsz) in enumerate(gl):
                if s == 0 and co <= slot_t0[g] < co + clen:
                    x_c[g] = (t, clen, slot_t0[g] - co)
            co += clen
        # b1 is pre-transposed on the host: a [P, 32] row-contiguous DMA.
        # (A "(g p) -> p g" rearrange here costs 4096 four-byte descriptors,
        # which jams the SDMA engines for ~7us right in the startup window.)
        b1_raw = bpool.tile([P, EA * NF_S], F32, name="b1r")
        nc.gpsimd.dma_start(b1_raw[:], b1_h[:, :])
        # Funnel b1 through an ACT-engine copy: downstream gelus then reach it
        # via same-engine program order instead of an extra semaphore wait.
        b1_t = bpool.tile([P, EA * NF_S], F32, name="b1c")
        nc.scalar.copy(b1_t[:], b1_raw[:])
        # warm output kept tiny and written once from the zero tile so the
        # "warm" DRAM tensor stays a valid output without a data-wait stall
        nc.sync.dma_start(warm_h[:, :], zpool_ws[:])

        # --- main loop: GEMM1(g) then GEMM2(g-1), pipelined so the gelu of
        # tile g's last f-block completes while GEMM2(g-1) occupies the PE.
        h_t = [None] * G
        y_defer = []

        def gemm2(j):
            s, _, tsz = gl[j]
            yst = ypool.tile([P, ND * T_TILE], BF16, tag="yst", name=f"y{j}")
            for dd in range(ND):
                pt2 = ps2.tile([P, T_TILE], F32, tag="ps2", name="pt2")
                for f in range(NF_S):
                    nc.tensor.matmul(
                        pt2[:, :tsz],
                        w2_block(s, dd, f),
                        h_t[j][f][:, :tsz],
                        start=(f == 0),
                        stop=(f == NF_S - 1),
                    )
                nc.vector.tensor_copy(
                    yst[:, dd * tsz:(dd + 1) * tsz], pt2[:, :tsz])
            # y stores ride the gpsimd (SWDGE) ring so they never contend
            # with x loads for a HWDGE ring slot; the final two tiles use the
            # by-then-idle HWDGE rings (shorter completion latency at the
            # tail), the last one split across both rings in parallel.
            if j == G - 1:
                nc.scalar.dma_start(
                    y_h[j * P:(j + 1) * P, :ND * tsz // 2],
                    yst[:, :ND * tsz // 2])
                nc.sync.dma_start(
                    y_h[j * P:(j + 1) * P, ND * tsz // 2:ND * tsz],
                    yst[:, ND * tsz // 2:ND * tsz])
            elif s == 0 and EA > 2:
                # Slot-0 partials are held in SBUF and stored later: their
                # ~1.5MB would otherwise steal SDMA engine time from the
                # startup-critical x/w fill.
                y_defer.append((j, yst, tsz))
            else:
                # Steady-state y stores ride the gpsimd (SWDGE) ring: its
                # sequencer waits never block the x/w prefetch issues on the
                # HWDGE rings (routing them there measured a 50us regression
                # -- each y's data-wait stalls the ring's later loads).  The
                # last few stores move to the scalar ring, whose load queue
                # is empty by then: the SWDGE queue's serial completions
                # otherwise lag past the final matmul and gate the drain.
                if j >= G - 6:
                    eng = nc.scalar if (G - 1 - j) % 2 else nc.sync
                else:
                    eng = nc.gpsimd
                eng.dma_start(
                    y_h[j * P:(j + 1) * P, :ND * tsz], yst[:, :ND * tsz])
            h_t[j] = None

        slot_ti = []
        cur = -1
        for s, _, _ in gl:
            if s != cur:
                cur, ti = s, 0
            slot_ti.append(ti)
            ti += 1
        for g, (s, first, tsz) in enumerate(gl):
            if first and s == 2:
                for (jd, ystd, tszd) in y_defer:
                    nc.gpsimd.dma_start(
                        y_h[jd * P:(jd + 1) * P, :ND * tszd],
                        ystd[:, :ND * tszd])
                y_defer = []
            if first and s + 1 < EA:
                # One whole-slot x DMA for the next slot, issued a full slot
                # (~25us) ahead of first use.
                t = dma_x_chunk(s + 1, 0, sum(slot_tiles[s + 1]))
                for g2, (s2, _, tsz2) in enumerate(gl):
                    if s2 == s + 1:
                        x_c[g2] = (t, sum(slot_tiles[s + 1]), slot_t0[g2])
            hs = [hpool.tile([P, T_TILE], BF16, tag=f"h{f}", name=f"h{g}_{f}")
                  for f in range(NF_S)]
            xt, clen, xo = x_c[g]
            for f in range(NF_S):
                pt = ps1.tile([P, T_TILE], F32, tag="ps1", name="pt1")
                for k in range(KD):
                    nc.tensor.matmul(
                        pt[:, :tsz],
                        w1_block(s, f, k),
                        xt[:, k * clen + xo:k * clen + xo + tsz],
                        start=(k == 0),
                        stop=(k == KD - 1),
                    )
                nc.scalar.activation(
                    hs[f][:, :tsz], pt[:, :tsz], gelu,
                    bias=b1_t[:, s * NF_S + f:s * NF_S + f + 1],
                )
                wlead = min(1, len(slot_tiles[s]) - 1)
                if f == 0 and slot_ti[g] == wlead and s + 1 < EA:
                    # Emit the next expert slot's weight loads behind the
                    # SECOND tile's gelu: the ACT ring is FIFO, so the 2MB
                    # weight transfer only starts once this slot's x tiles
                    # have cleared -- it can't starve the startup-critical
                    # fill, and still lands a full slot (~20us) early.
                    w_t[s + 1] = dma_w(s + 1, split=False)
            h_t[g] = hs
            if g < 4:
                warm_fill((10, 10, 6, 4)[g])
            if g >= PIPE:
                gemm2(g - PIPE)
                if g - PIPE < 3:
                    warm_fill((8, 6, 4)[g - PIPE])
        for j in range(max(0, G - PIPE), G):
            gemm2(j)
    return _patch_to_json(nc)


def _route(xf: np.ndarray, gate_w: np.ndarray):
    """Top-2 gating identical to the reference (argmax ties -> lower index)."""
    N = xf.shape[0]
    logits = xf @ gate_w  # (N, E) f32
    rows = np.arange(N)
    i1 = logits.argmax(1)
    v1 = logits[rows, i1]
    masked = logits.copy()
    masked[rows, i1] = -np.inf
    i2 = masked.argmax(1)
    v2 = masked[rows, i2]
    # softmax over the two selected logits (v1 >= v2)
    e = np.exp((v2 - v1).astype(np.float32))
    wt1 = (1.0 / (1.0 + e)).astype(np.float32)
    wt2 = (e / (1.0 + e)).astype(np.float32)
    idx_e, wts_e = [], []
    for ex in range(E):
        s1 = np.nonzero(i1 == ex)[0]
        s2 = np.nonzero(i2 == ex)[0]
        idx_e.append(np.concatenate([s1, s2]))
        wts_e.append(np.concatenate([wt1[s1], wt2[s2]]).astype(np.float32))
    return idx_e, wts_e


def kernel(x, gate_w, w1, b1, w2, b2, _trace=False):
    B, T, D_ = x.shape
    N = B * T
    xf = np.ascontiguousarray(x.reshape(N, D_).astype(np.float32))
    idx_e, wts_e = _route(xf, gate_w.astype(np.float32))
    cnts = np.array([len(i) for i in idx_e])
    order = np.argsort(-cnts, kind="stable")
    order = [int(e) for e in order if cnts[e] > 0]
    slot_tiles = [_t_sizes(int(cnts[e]), small_tail=(s == len(order) - 1),
                           ramp=(s == 0))
                  for s, e in enumerate(order)]
    EA = len(order)
    gl = []  # (slot, t0_within_expert, tsz)
    for s, sizes in enumerate(slot_tiles):
        t0 = 0
        for tsz in sizes:
            gl.append((s, t0, tsz))
            t0 += tsz
    G = len(gl)

    key = tuple(tuple(st) for st in slot_tiles)
    if key in _neff_cache:
        nc = _neff_cache[key]
    else:
        nc = _build_bass(slot_tiles)
        _neff_cache[key] = nc

    # --- host-side pre-blocking (shared across cores for xs, per-core for w)
    chunks = _x_chunks(slot_tiles)
    LMAX = max(sum(c) for c in chunks)
    xs = np.zeros((EA * P, KD * LMAX), NP_BF16)
    xg_by_slot = [xf[idx_e[e]] for e in order]
    for s, cl in enumerate(chunks):
        co = 0
        for clen in cl:
            blk = xg_by_slot[s][co:co + clen]                  # [clen, D] f32
            b3 = blk.T.reshape(KD, P, clen).transpose(1, 0, 2)  # [P, KD, clen]
            xs[s * P:(s + 1) * P, KD * co:KD * (co + clen)] = (
                b3.reshape(P, KD * clen).astype(NP_BF16))
            co += clen

    in_maps = []
    for c in range(8):
        cS = c * S
        w1s = np.empty((EA * P, NF_S * KD * P), NP_BF16)
        w2s = np.empty((EA * P, NF_S * ND * P), NP_BF16)
        b1s = np.empty((P, EA * NF_S), np.float32)
        for s, e in enumerate(order):
            a = w1[e][:, cS:cS + S]                          # [D, S]
            w1s[s * P:(s + 1) * P] = (
                a.reshape(KD, P, NF_S, P).transpose(1, 2, 0, 3)
                .reshape(P, NF_S * KD * P).astype(NP_BF16))
            bslc = w2[e][cS:cS + S, :]                       # [S, D]
            w2s[s * P:(s + 1) * P] = (
                bslc.reshape(NF_S, P, ND, P).transpose(1, 2, 0, 3)
                .reshape(P, ND * NF_S * P).astype(NP_BF16))
            b1s[:, s * NF_S:(s + 1) * NF_S] = (
                b1[e][cS:cS + S].reshape(NF_S, P).T)
        in_maps.append({
            "xs": xs,
            "w1": w1s,
            "w2": w2s,
            "b1": np.ascontiguousarray(b1s),
        })

    res = run_bass_kernel_spmd(nc, in_maps, core_ids=list(range(8)),
                               trace=_trace)
    if _trace:
        print(f"HW exec time: {res.exec_time_ns} ns")

    # --- unshard: sum the 8 dff-slice partials, then combine + scatter-add
    ysum = np.zeros((G * P, ND * T_TILE), np.float32)
    for c in range(8):
        ysum += res.results[c]["y"].astype(np.float32)

    out = np.zeros((N, D), np.float32)
    for s, e in enumerate(order):
        L = int(cnts[e])
        ye = np.empty((L, D), np.float32)
        for g, (sg, t0, tsz) in enumerate(gl):
            if sg != s:
                continue
            blk = ysum[g * P:(g + 1) * P, :ND * tsz]
            ye[t0:t0 + tsz] = (
                blk.reshape(P, ND, tsz).transpose(2, 1, 0).reshape(tsz, D))
        yv = ye + b2[e][None, :].astype(np.float32)
        out[idx_e[e]] += wts_e[e][:, None] * yv
    return out.reshape(B, T, D_)
